# revision 30
# baseline (speedup 1.0000x reference)
"""NeLDA loglik kernel for 8 Trainium2 NeuronCores (Bass/Tile).

Strategy: vocab-parallel over V=50000 (6250 cols/core, padded to 6272).
  phase 1: per-core partial fc1 (contraction over its V-slice), AllReduce
           the [H=100, B=1024] pre-activation.
  phase 2: replicated small net (fc2, mu/sig BatchNorm, theta) in
           transposed [feature, batch] layout.
  phase 3: per-core logits slice x = theta @ beta_w[:, slice] (in PSUM,
           [v, b] layout), BatchNorm stats per v-row via bn_stats,
           exp fused with BN via ScalarE per-partition scale/bias,
           partition-dir reductions (sum_v exp, sum_v cnt*y) via
           ones/counts matmuls packed on PE column strips.
  final:   AllReduce [sumexp | S] (8KB), loglik = S - T*ln(sumexp).

The token gather sum_t log_beta[:, tok_t] is rewritten as a count-weighted
sum over the vocab: S[b] = sum_v cnt_v * y[v, b], cnt = histogram(tokens).
"""

import ml_dtypes
import numpy as np

import concourse.bass as bass
import concourse.mybir as mybir
import concourse.tile as tile
from concourse import bacc
from concourse.bass_utils import run_bass_kernel_spmd

F32 = mybir.dt.float32
F32R = mybir.dt.float32r
BF16 = mybir.dt.bfloat16
AF = mybir.ActivationFunctionType
OP = mybir.AluOpType

N_CORES = 8
B = 1024
V = 50000
H = 100
K = 200
T_TOK = 32768
BN_EPS = 1e-5

DEBUG = False
TRUNC = 0  # 1: stop after AR1, 2: stop after small net
# f32r (TF32-like, full-rate) for the big beta matmul; exact f32 for fc1 and
# the small net (fc1 in f32r triggers a hardware fault — see notes).
PH1_DT = F32
PH3_DT = F32R
SN_DT = F32

VS = V // N_CORES          # 6250 per-core vocab slice
NCH = (VS + 127) // 128    # 49 chunks of 128 v-rows
VP = NCH * 128             # 6272 padded
DMA_GRP = 2                # bows chunks per DMA (1 MiB each)


def _build_nc():
    nc = bacc.Bacc("TRN2", target_bir_lowering=False, num_devices=N_CORES)

    bowsT = nc.dram_tensor("bowsT", [VP, B], PH1_DT, kind="ExternalInput")
    w1 = nc.dram_tensor("w1", [128, NCH * H], PH1_DT, kind="ExternalInput")
    beta0 = nc.dram_tensor("beta0", [H, VP], PH3_DT, kind="ExternalInput")
    beta1 = nc.dram_tensor("beta1", [H, VP], PH3_DT, kind="ExternalInput")
    epst = nc.dram_tensor("epst", [H, 2 * B], F32, kind="ExternalInput")
    fc1b = nc.dram_tensor("fc1b", [H, 1], F32, kind="ExternalInput")
    fc2w = nc.dram_tensor("fc2w", [H, H], SN_DT, kind="ExternalInput")
    fc2b = nc.dram_tensor("fc2b", [H, 1], F32, kind="ExternalInput")
    muw = nc.dram_tensor("muw", [H, K], SN_DT, kind="ExternalInput")
    sigw = nc.dram_tensor("sigw", [H, K], SN_DT, kind="ExternalInput")
    bnmg = nc.dram_tensor("bnmg", [H, 2], F32, kind="ExternalInput")
    bnmb = nc.dram_tensor("bnmb", [H, 2], F32, kind="ExternalInput")
    bnsg = nc.dram_tensor("bnsg", [H, 2], F32, kind="ExternalInput")
    bnsb = nc.dram_tensor("bnsb", [H, 2], F32, kind="ExternalInput")
    vg = nc.dram_tensor("vg", [128, NCH], F32, kind="ExternalInput")
    vb = nc.dram_tensor("vb", [128, NCH], F32, kind="ExternalInput")
    vbb = nc.dram_tensor("vbb", [128, NCH], F32, kind="ExternalInput")
    cnt = nc.dram_tensor("cnt", [128, NCH], BF16, kind="ExternalInput")
    msk = nc.dram_tensor("msk", [128, NCH], BF16, kind="ExternalInput")

    loglik = nc.dram_tensor("loglik", [1, B], F32, kind="ExternalOutput")
    if DEBUG:
        dbg_theta = nc.dram_tensor("dbg_theta", [H, 2 * B], F32, kind="ExternalOutput")
        dbg_stg = nc.dram_tensor("dbg_stg", [128, 512], F32, kind="ExternalOutput")
        dbg_fin = nc.dram_tensor("dbg_fin", [1, 2 * B], F32, kind="ExternalOutput")
        dbg_h2 = nc.dram_tensor("dbg_h2", [H, B], F32, kind="ExternalOutput")

    cc1_in = nc.dram_tensor("cc1_in", [H, B], F32)
    cc1_out = nc.dram_tensor("cc1_out", [H, B], F32, addr_space="Shared")
    cc2_in = nc.dram_tensor("cc2_in", [1, 2 * B], F32)
    cc2_out = nc.dram_tensor("cc2_out", [1, 2 * B], F32, addr_space="Shared")

    rg = [list(range(N_CORES))]

    bows_v = bowsT.rearrange("(c p) b -> c p b", p=128)  # [NCH, 128, B]

    with tile.TileContext(nc) as tc:
        with (
            tc.tile_pool(name="consts", bufs=1) as consts,
            tc.tile_pool(name="work", bufs=3) as work,
            tc.tile_pool(name="small", bufs=4) as small,
        ):
            # ---- resident loads -------------------------------------------
            w1_sb = consts.tile([128, NCH * H], PH1_DT)
            nc.sync.dma_start(out=w1_sb, in_=w1[:, :])
            b0_sb = consts.tile([H, VP], PH3_DT)
            nc.sync.dma_start(out=b0_sb, in_=beta0[:, :])
            b1_sb = consts.tile([H, VP], PH3_DT)
            nc.sync.dma_start(out=b1_sb, in_=beta1[:, :])
            epst_sb = consts.tile([H, 2 * B], F32)
            nc.sync.dma_start(out=epst_sb, in_=epst[:, :])
            fc1b_sb = consts.tile([H, 1], F32)
            nc.sync.dma_start(out=fc1b_sb, in_=fc1b[:, :])
            fc2w_sb = consts.tile([H, H], SN_DT)
            nc.sync.dma_start(out=fc2w_sb, in_=fc2w[:, :])
            fc2b_sb = consts.tile([H, 1], F32)
            nc.sync.dma_start(out=fc2b_sb, in_=fc2b[:, :])
            muw_sb = consts.tile([H, K], SN_DT)
            nc.sync.dma_start(out=muw_sb, in_=muw[:, :])
            sigw_sb = consts.tile([H, K], SN_DT)
            nc.sync.dma_start(out=sigw_sb, in_=sigw[:, :])
            bnmg_sb = consts.tile([H, 2], F32)
            nc.sync.dma_start(out=bnmg_sb, in_=bnmg[:, :])
            bnmb_sb = consts.tile([H, 2], F32)
            nc.sync.dma_start(out=bnmb_sb, in_=bnmb[:, :])
            bnsg_sb = consts.tile([H, 2], F32)
            nc.sync.dma_start(out=bnsg_sb, in_=bnsg[:, :])
            bnsb_sb = consts.tile([H, 2], F32)
            nc.sync.dma_start(out=bnsb_sb, in_=bnsb[:, :])
            vg_sb = consts.tile([128, NCH], F32)
            nc.sync.dma_start(out=vg_sb, in_=vg[:, :])
            vb_sb = consts.tile([128, NCH], F32)
            nc.sync.dma_start(out=vb_sb, in_=vb[:, :])
            vbb_sb = consts.tile([128, NCH], F32)
            nc.sync.dma_start(out=vbb_sb, in_=vbb[:, :])
            cnt_sb = consts.tile([128, NCH], BF16)
            nc.sync.dma_start(out=cnt_sb, in_=cnt[:, :])
            msk_sb = consts.tile([128, NCH], BF16)
            nc.sync.dma_start(out=msk_sb, in_=msk[:, :])
            epsc_h = consts.tile([H, 1], F32)
            nc.vector.memset(epsc_h, BN_EPS)
            epsc_128 = consts.tile([128, 1], F32)
            nc.vector.memset(epsc_128, BN_EPS)

            # ---- phase 1: fc1 partial over the V-slice --------------------
            sn_pool_cm = tc.tile_pool(name="sn", bufs=1)
            sn = sn_pool_cm.__enter__()
            with tc.tile_pool(name="ph1", bufs=2, space="PSUM") as ph1:
                h1a = ph1.tile([H, 512], F32, tag="h1")
                h1b = ph1.tile([H, 512], F32, tag="h1")
                n_grp = (NCH + DMA_GRP - 1) // DMA_GRP
                for d in range(n_grp):
                    j0 = d * DMA_GRP
                    nch = min(DMA_GRP, NCH - j0)
                    bt = work.tile([128, DMA_GRP * B], PH1_DT, tag="bows")
                    bt3 = bt.rearrange("p (c b) -> p c b", c=DMA_GRP)
                    nc.sync.dma_start(
                        out=bt3[:, :nch, :],
                        in_=bows_v[j0 : j0 + nch].rearrange("c p b -> p c b"),
                    )
                    for j in range(nch):
                        k = j0 + j
                        lhs = w1_sb[:, k * H : (k + 1) * H]
                        nc.tensor.matmul(
                            h1a[:, :],
                            lhs,
                            bt3[:, j, 0:512],
                            start=(k == 0),
                            stop=(k == NCH - 1),
                        )
                        nc.tensor.matmul(
                            h1b[:, :],
                            lhs,
                            bt3[:, j, 512:1024],
                            start=(k == 0),
                            stop=(k == NCH - 1),
                        )
                h1pre = sn.tile([H, B], F32)
                nc.vector.tensor_copy(h1pre[:, 0:512], h1a[:, :])
                nc.vector.tensor_copy(h1pre[:, 512:1024], h1b[:, :])
                nc.sync.dma_start(out=cc1_in[:, :], in_=h1pre)

            nc.gpsimd.collective_compute(
                "AllReduce", OP.add, replica_groups=rg,
                ins=[cc1_in[:, :]], outs=[cc1_out[:, :]],
            )

            # ---- phase 2: replicated small net ----------------------------
            har = sn.tile([H, B], F32)
            nc.sync.dma_start(out=har, in_=cc1_out[:, :])
            if TRUNC == 1:
                nc.sync.dma_start(out=loglik[:, :], in_=har[0:1, :])
                nc.compile_marker_trunc = True
            # softplus(x) = ln(1 + exp(x)); |x| < ~15 so no overflow concern.
            if TRUNC == 1:
                return _finish(nc)
            h1e = sn.tile([H, B], F32)
            nc.scalar.activation(h1e, har, AF.Exp, bias=fc1b_sb, scale=1.0)
            h1_bf = sn.tile([H, B], SN_DT)
            nc.scalar.activation(h1_bf, h1e, AF.Ln, bias=1.0, scale=1.0)

            theta_bf = consts.tile([H, 2 * B], PH3_DT)
            mun = sn.tile([H, 2 * B], F32)
            sige = sn.tile([H, 2 * B], F32)
            h2_bf = sn.tile([H, B], SN_DT)

            with tc.tile_pool(name="psn", bufs=2, space="PSUM") as psn:
                for half in range(2):
                    hps = psn.tile([H, 512], F32, tag="sn")
                    nc.tensor.matmul(
                        hps, fc2w_sb, h1_bf[:, half * 512 : (half + 1) * 512],
                        start=True, stop=True,
                    )
                    h2e = small.tile([H, 512], F32, tag="h2e", bufs=2)
                    nc.scalar.activation(h2e, hps, AF.Exp, bias=fc2b_sb, scale=1.0)
                    nc.scalar.activation(
                        h2_bf[:, half * 512 : (half + 1) * 512],
                        h2e, AF.Ln, bias=1.0, scale=1.0,
                    )

                for kh in range(2):
                    for w_sb, gg, bb_, dst, is_sig in (
                        (muw_sb, bnmg_sb, bnmb_sb, mun, False),
                        (sigw_sb, bnsg_sb, bnsb_sb, sige, True),
                    ):
                        pa = psn.tile([H, 512], F32, tag="sn")
                        pb = psn.tile([H, 512], F32, tag="sn")
                        lhs = w_sb[:, kh * H : (kh + 1) * H]
                        nc.tensor.matmul(pa, lhs, h2_bf[:, 0:512], start=True, stop=True)
                        nc.tensor.matmul(pb, lhs, h2_bf[:, 512:1024], start=True, stop=True)
                        st = small.tile([H, 2, 6], F32, tag="snst")
                        nc.vector.bn_stats(st[:, 0, :], pa)
                        nc.vector.bn_stats(st[:, 1, :], pb)
                        mv = small.tile([H, 2], F32, tag="snmv")
                        nc.vector.bn_aggr(mv, st)
                        # rstd = exp(-0.5*ln(var+eps)); keeps ACT on one table set
                        lnu = small.tile([H, 1], F32, tag="snsq")
                        nc.scalar.activation(lnu, mv[:, 1:2], AF.Ln, bias=epsc_h, scale=1.0)
                        rstd = small.tile([H, 1], F32, tag="snr")
                        nc.scalar.activation(rstd, lnu, AF.Exp, bias=0.0, scale=-0.5)
                        gr = small.tile([H, 1], F32, tag="sngr")
                        nc.vector.tensor_mul(gr, gg[:, kh : kh + 1], rstd)
                        if is_sig:
                            # sige = exp(0.5*(g*rstd*(x-m)+b)) fused on ACT
                            sc = small.tile([H, 1], F32, tag="snsc")
                            nc.vector.tensor_scalar_mul(sc, gr, 0.5)
                            bi = small.tile([H, 1], F32, tag="snbi")
                            nc.vector.tensor_mul(bi, sc, mv[:, 0:1])
                            bi2 = small.tile([H, 1], F32, tag="snbi2")
                            nc.vector.tensor_scalar_mul(bi2, bb_[:, kh : kh + 1], 0.5)
                            bi3 = small.tile([H, 1], F32, tag="snbi3")
                            nc.vector.tensor_sub(bi3, bi2, bi)
                            for half, ps in ((0, pa), (1, pb)):
                                nc.scalar.activation(
                                    dst[:, kh * B + half * 512 : kh * B + (half + 1) * 512],
                                    ps, AF.Exp, bias=bi3, scale=sc,
                                )
                        else:
                            # mun = (x - m) * (g*rstd)   (bias bnmb added later)
                            for half, ps in ((0, pa), (1, pb)):
                                nc.vector.tensor_scalar(
                                    dst[:, kh * B + half * 512 : kh * B + (half + 1) * 512],
                                    ps, mv[:, 0:1], gr, op0=OP.subtract, op1=OP.mult,
                                )

                # theta = exp(mun + bnmb + sige * epsT)
                for kh in range(2):
                    for half in range(2):
                        s = slice(kh * B + half * 512, kh * B + (half + 1) * 512)
                        tmp = small.tile([H, 512], F32, tag="snth", bufs=2)
                        nc.vector.tensor_mul(tmp, sige[:, s], epst_sb[:, s])
                        tmp2 = small.tile([H, 512], F32, tag="snth2", bufs=2)
                        nc.vector.tensor_add(tmp2, tmp, mun[:, s])
                        nc.scalar.activation(
                            theta_bf[:, s], tmp2, AF.Exp,
                            bias=bnmb_sb[:, kh : kh + 1], scale=1.0,
                        )

            sn_pool_cm.__exit__(None, None, None)

            if TRUNC == 2:
                trsb = consts.tile([1, B], F32)
                nc.vector.tensor_copy(trsb, theta_bf[0:1, 0:B])
                nc.sync.dma_start(out=loglik[:, :], in_=trsb)
                return _finish(nc)

            # ---- phase 3: beta matmul + BN + exp + reductions -------------
            with (
                tc.tile_pool(name="px", bufs=3, space="PSUM") as px,
                tc.tile_pool(name="pred", bufs=1, space="PSUM") as pred,
                tc.tile_pool(name="ph3w", bufs=1) as ph3w,
            ):
                red = pred.tile([128, 512], F32)
                PIPE = 6  # reduction matmuls trail the x matmuls by PIPE chunks
                pend = {}

                def emit_red(j):
                    ea, ya = pend.pop(j)
                    mcol = msk_sb[:, j : j + 1]
                    ccol = cnt_sb[:, j : j + 1]
                    st_ = (j == 0)
                    sp_ = (j == NCH - 1)
                    nc.tensor.matmul(red[0:1, :], mcol, ea[:, 0:512], start=st_,
                                     stop=sp_, tile_position=(0, 0))
                    nc.tensor.matmul(red[32:33, :], mcol, ea[:, 512:1024], start=st_,
                                     stop=sp_, tile_position=(0, 32))
                    nc.tensor.matmul(red[64:65, :], ccol, ya[:, 0:512], start=st_,
                                     stop=sp_, tile_position=(0, 64))
                    nc.tensor.matmul(red[96:97, :], ccol, ya[:, 512:1024], start=st_,
                                     stop=sp_, tile_position=(0, 96))

                for j in range(NCH):
                    vs = slice(j * 128, (j + 1) * 128)
                    xp = px.tile([128, 1024], F32, tag="x")
                    nc.tensor.matmul(xp[:, 0:512], b0_sb[:, vs], theta_bf[:, 0:512], start=True, stop=False)
                    nc.tensor.matmul(xp[:, 0:512], b1_sb[:, vs], theta_bf[:, B : B + 512], start=False, stop=True)
                    nc.tensor.matmul(xp[:, 512:1024], b0_sb[:, vs], theta_bf[:, 512:1024], start=True, stop=False)
                    nc.tensor.matmul(xp[:, 512:1024], b1_sb[:, vs], theta_bf[:, B + 512 : 2 * B], start=False, stop=True)

                    st = small.tile([128, 2, 6], F32, tag="xst")
                    nc.vector.bn_stats(st[:, 0, :], xp[:, 0:512])
                    nc.vector.bn_stats(st[:, 1, :], xp[:, 512:1024])
                    mv = small.tile([128, 2], F32, tag="xmv")
                    nc.vector.bn_aggr(mv, st)
                    # m2n = -mean; a = g*rsqrt(var+eps); c2 = a*m2n + bn_b
                    m2n = small.tile([128, 1], F32, tag="xm2")
                    nc.vector.tensor_scalar_mul(m2n, mv[:, 0:1], -1.0)
                    lnu = small.tile([128, 1], F32, tag="xsq")
                    nc.scalar.activation(lnu, mv[:, 1:2], AF.Ln, bias=epsc_128, scale=1.0)
                    rstd = small.tile([128, 1], F32, tag="xr")
                    nc.scalar.activation(rstd, lnu, AF.Exp, bias=0.0, scale=-0.5)
                    a = small.tile([128, 1], F32, tag="xa")
                    nc.vector.tensor_mul(a, vg_sb[:, j : j + 1], rstd)
                    c2 = small.tile([128, 1], F32, tag="xc2")
                    nc.vector.scalar_tensor_tensor(c2, a, m2n, vb_sb[:, j : j + 1],
                                                   op0=OP.mult, op1=OP.add)

                    ea = ph3w.tile([128, 1024], BF16, tag="ea", bufs=PIPE + 2)
                    nc.scalar.activation(ea, xp, AF.Exp, bias=c2, scale=a)
                    ya = ph3w.tile([128, 1024], BF16, tag="ya", bufs=PIPE + 2)
                    nc.scalar.activation(ya[:, 0:512], xp[:, 0:512], AF.Identity,
                                         bias=c2, scale=a)
                    nc.vector.tensor_scalar(ya[:, 512:1024], xp[:, 512:1024], a, c2,
                                            op0=OP.mult, op1=OP.add)
                    pend[j] = (ea, ya)
                    if j >= PIPE:
                        emit_red(j - PIPE)
                for j in range(NCH - PIPE, NCH):
                    emit_red(j)

                stg = ph3w.tile([128, 512], F32)
                nc.vector.tensor_copy(stg, red[:, :])
                if DEBUG:
                    nc.sync.dma_start(out=dbg_theta[:, :], in_=theta_bf)
                    nc.sync.dma_start(out=dbg_stg[:, :], in_=stg)
                    nc.sync.dma_start(out=dbg_h2[:, :], in_=h2_bf)
                nc.sync.dma_start(out=cc2_in[0:1, 0:512], in_=stg[0:1, :])
                nc.sync.dma_start(out=cc2_in[0:1, 512:1024], in_=stg[32:33, :])
                nc.sync.dma_start(out=cc2_in[0:1, 1024:1536], in_=stg[64:65, :])
                nc.sync.dma_start(out=cc2_in[0:1, 1536:2048], in_=stg[96:97, :])

            nc.gpsimd.collective_compute(
                "AllReduce", OP.add, replica_groups=rg,
                ins=[cc2_in[:, :]], outs=[cc2_out[:, :]],
            )

            # ---- final: loglik = S - T*ln(sumexp) -------------------------
            finp_cm = tc.tile_pool(name="finp", bufs=1)
            finp = finp_cm.__enter__()
            fin = finp.tile([1, 2 * B], F32)
            nc.sync.dma_start(out=fin, in_=cc2_out[:, :])
            if DEBUG:
                nc.sync.dma_start(out=dbg_fin[:, :], in_=fin)
            lse = finp.tile([1, B], F32)
            nc.scalar.activation(lse, fin[:, 0:B], AF.Ln, bias=0.0, scale=1.0)
            lt = finp.tile([1, B], F32)
            nc.vector.tensor_scalar_mul(lt, lse, float(T_TOK))
            res = finp.tile([1, B], F32)
            nc.vector.tensor_sub(res, fin[:, B : 2 * B], lt)
            nc.sync.dma_start(out=loglik[:, :], in_=res)
            finp_cm.__exit__(None, None, None)

    return _finish(nc)


def _finish(nc):
    nc.compile()
    return nc


_NC_CACHE = None


def _get_nc():
    global _NC_CACHE
    if _NC_CACHE is None:
        _NC_CACHE = _build_nc()
    return _NC_CACHE


def _np_dt(mdt):
    import concourse.mybir as _mb
    return ml_dtypes.bfloat16 if mdt == _mb.dt.bfloat16 else np.float32


def _prep_core_inputs(c, bows, eps, fc1_w, fc1_b, fc2_w, fc2_b, mu_w,
                      bn_mu_g, bn_mu_b, sig_w, bn_sig_g, bn_sig_b,
                      beta_w, beta_b, bn_beta_g, bn_beta_b, counts):
    sl = slice(c * VS, (c + 1) * VS)

    def padv(x, fill=0.0):
        out = np.full(VP, fill, np.float32)
        out[:VS] = x[sl]
        return out

    bowsT = np.zeros((VP, B), np.float32)
    bowsT[:VS] = bows[:, sl].T
    w1 = np.zeros((VP, H), np.float32)
    w1[:VS] = fc1_w[sl]

    vcol = lambda x: np.ascontiguousarray(x.reshape(NCH, 128).T)

    return {
        "bowsT": bowsT.astype(_np_dt(PH1_DT)),
        "w1": np.ascontiguousarray(
            w1.reshape(NCH, 128, H).transpose(1, 0, 2).reshape(128, NCH * H)
        ).astype(_np_dt(PH1_DT)),
        "beta0": np.ascontiguousarray(
            np.pad(beta_w[0:H, sl], ((0, 0), (0, VP - VS)))
        ).astype(_np_dt(PH3_DT)),
        "beta1": np.ascontiguousarray(
            np.pad(beta_w[H:K, sl], ((0, 0), (0, VP - VS)))
        ).astype(_np_dt(PH3_DT)),
        "epst": np.ascontiguousarray(
            eps.T.reshape(2, H, B).transpose(1, 0, 2).reshape(H, 2 * B)
        ).astype(np.float32),
        "fc1b": fc1_b.reshape(H, 1).astype(np.float32),
        "fc2w": fc2_w.astype(_np_dt(SN_DT)),
        "fc2b": fc2_b.reshape(H, 1).astype(np.float32),
        "muw": np.ascontiguousarray(mu_w.reshape(H, K)).astype(_np_dt(SN_DT)),
        "sigw": np.ascontiguousarray(sig_w.reshape(H, K)).astype(_np_dt(SN_DT)),
        "bnmg": np.ascontiguousarray(bn_mu_g.reshape(2, H).T).astype(np.float32),
        "bnmb": np.ascontiguousarray(bn_mu_b.reshape(2, H).T).astype(np.float32),
        "bnsg": np.ascontiguousarray(bn_sig_g.reshape(2, H).T).astype(np.float32),
        "bnsb": np.ascontiguousarray(bn_sig_b.reshape(2, H).T).astype(np.float32),
        "vg": vcol(padv(bn_beta_g, 1.0)),
        "vb": vcol(padv(bn_beta_b)),
        "vbb": vcol(padv(beta_b)),
        "cnt": vcol(padv(counts)).astype(ml_dtypes.bfloat16),
        "msk": vcol(np.pad(np.ones(VS, np.float32), (0, VP - VS))).astype(ml_dtypes.bfloat16),
    }


def kernel(bows, eps, ne_tokens, fc1_w, fc1_b, fc2_w, fc2_b,
           mu_w, mu_b, bn_mu_g, bn_mu_b, sig_w, sig_b, bn_sig_g, bn_sig_b,
           beta_w, beta_b, bn_beta_g, bn_beta_b):
    bows = np.asarray(bows, np.float32)
    eps = np.asarray(eps, np.float32)
    counts = np.bincount(np.asarray(ne_tokens), minlength=V).astype(np.float32)

    # mu_b / sig_b cancel inside BatchNorm (shift-invariant); not shipped.
    args = (bows, eps, np.asarray(fc1_w, np.float32), np.asarray(fc1_b, np.float32),
            np.asarray(fc2_w, np.float32), np.asarray(fc2_b, np.float32),
            np.asarray(mu_w, np.float32), np.asarray(bn_mu_g, np.float32),
            np.asarray(bn_mu_b, np.float32), np.asarray(sig_w, np.float32),
            np.asarray(bn_sig_g, np.float32), np.asarray(bn_sig_b, np.float32),
            np.asarray(beta_w, np.float32), np.asarray(beta_b, np.float32),
            np.asarray(bn_beta_g, np.float32), np.asarray(bn_beta_b, np.float32),
            counts)

    in_maps = [_prep_core_inputs(c, *args) for c in range(N_CORES)]
    global _last_in_maps
    _last_in_maps = in_maps
    nc = _get_nc()
    res = run_bass_kernel_spmd(nc, in_maps, list(range(N_CORES)))
    return np.asarray(res.results[0]["loglik"]).reshape(B).astype(np.float32)


_last_in_maps = None


# revision 32
# speedup vs baseline: 1.0584x; 1.0584x over previous
"""NeLDA loglik kernel for 8 Trainium2 NeuronCores (Bass/Tile).

Strategy: vocab-parallel over V=50000 (6250 cols/core, padded to 6272).
  phase 1: per-core partial fc1 (contraction over its V-slice), AllReduce
           the [H=100, B=1024] pre-activation.
  phase 2: replicated small net (fc2, mu/sig BatchNorm, theta) in
           transposed [feature, batch] layout.
  phase 3: per-core logits slice x = theta @ beta_w[:, slice] (in PSUM,
           [v, b] layout), BatchNorm stats per v-row via bn_stats,
           exp fused with BN via ScalarE per-partition scale/bias,
           partition-dir reductions (sum_v exp, sum_v cnt*y) via
           ones/counts matmuls packed on PE column strips.
  final:   AllReduce [sumexp | S] (8KB), loglik = S - T*ln(sumexp).

The token gather sum_t log_beta[:, tok_t] is rewritten as a count-weighted
sum over the vocab: S[b] = sum_v cnt_v * y[v, b], cnt = histogram(tokens).
"""

import ml_dtypes
import numpy as np

import concourse.bass as bass
import concourse.mybir as mybir
import concourse.tile as tile
from concourse import bacc
from concourse.bass_utils import run_bass_kernel_spmd

F32 = mybir.dt.float32
F32R = mybir.dt.float32r
BF16 = mybir.dt.bfloat16
AF = mybir.ActivationFunctionType
OP = mybir.AluOpType

N_CORES = 8
B = 1024
V = 50000
H = 100
K = 200
T_TOK = 32768
BN_EPS = 1e-5

DEBUG = False
YA_ON_ACT = True
TRUNC = 0  # 1: stop after AR1, 2: stop after small net
# f32r (TF32-like, full-rate) for the big beta matmul; exact f32 for fc1 and
# the small net (fc1 in f32r triggers a hardware fault — see notes).
PH1_DT = F32
PH3_DT = F32R
SN_DT = F32

VS = V // N_CORES          # 6250 per-core vocab slice
NCH = (VS + 127) // 128    # 49 chunks of 128 v-rows
VP = NCH * 128             # 6272 padded
DMA_GRP = 2                # bows chunks per DMA (1 MiB each)


def _build_nc():
    nc = bacc.Bacc("TRN2", target_bir_lowering=False, num_devices=N_CORES)

    bowsT = nc.dram_tensor("bowsT", [VP, B], PH1_DT, kind="ExternalInput")
    w1 = nc.dram_tensor("w1", [128, NCH * H], PH1_DT, kind="ExternalInput")
    beta0 = nc.dram_tensor("beta0", [H, VP], PH3_DT, kind="ExternalInput")
    beta1 = nc.dram_tensor("beta1", [H, VP], PH3_DT, kind="ExternalInput")
    epst = nc.dram_tensor("epst", [H, 2 * B], F32, kind="ExternalInput")
    fc1b = nc.dram_tensor("fc1b", [H, 1], F32, kind="ExternalInput")
    fc2w = nc.dram_tensor("fc2w", [H, H], SN_DT, kind="ExternalInput")
    fc2b = nc.dram_tensor("fc2b", [H, 1], F32, kind="ExternalInput")
    muw = nc.dram_tensor("muw", [H, K], SN_DT, kind="ExternalInput")
    sigw = nc.dram_tensor("sigw", [H, K], SN_DT, kind="ExternalInput")
    bnmg = nc.dram_tensor("bnmg", [H, 2], F32, kind="ExternalInput")
    bnmb = nc.dram_tensor("bnmb", [H, 2], F32, kind="ExternalInput")
    bnsg = nc.dram_tensor("bnsg", [H, 2], F32, kind="ExternalInput")
    bnsb = nc.dram_tensor("bnsb", [H, 2], F32, kind="ExternalInput")
    vg = nc.dram_tensor("vg", [128, NCH], F32, kind="ExternalInput")
    vb = nc.dram_tensor("vb", [128, NCH], F32, kind="ExternalInput")
    vbb = nc.dram_tensor("vbb", [128, NCH], F32, kind="ExternalInput")
    cnt = nc.dram_tensor("cnt", [128, NCH], BF16, kind="ExternalInput")
    msk = nc.dram_tensor("msk", [128, NCH], BF16, kind="ExternalInput")

    loglik = nc.dram_tensor("loglik", [1, B], F32, kind="ExternalOutput")
    if DEBUG:
        dbg_theta = nc.dram_tensor("dbg_theta", [H, 2 * B], F32, kind="ExternalOutput")
        dbg_stg = nc.dram_tensor("dbg_stg", [128, 512], F32, kind="ExternalOutput")
        dbg_fin = nc.dram_tensor("dbg_fin", [1, 2 * B], F32, kind="ExternalOutput")
        dbg_h2 = nc.dram_tensor("dbg_h2", [H, B], F32, kind="ExternalOutput")

    cc1_in = nc.dram_tensor("cc1_in", [H, B], F32)
    cc1_out = nc.dram_tensor("cc1_out", [H, B], F32, addr_space="Shared")
    cc2_in = nc.dram_tensor("cc2_in", [1, 2 * B], F32)
    cc2_out = nc.dram_tensor("cc2_out", [1, 2 * B], F32, addr_space="Shared")

    rg = [list(range(N_CORES))]

    bows_v = bowsT.rearrange("(c p) b -> c p b", p=128)  # [NCH, 128, B]

    with tile.TileContext(nc) as tc:
        with (
            tc.tile_pool(name="consts", bufs=1) as consts,
            tc.tile_pool(name="work", bufs=3) as work,
            tc.tile_pool(name="small", bufs=4) as small,
        ):
            # ---- resident loads -------------------------------------------
            w1_sb = consts.tile([128, NCH * H], PH1_DT)
            nc.sync.dma_start(out=w1_sb, in_=w1[:, :])
            b0_sb = consts.tile([H, VP], PH3_DT)
            nc.sync.dma_start(out=b0_sb, in_=beta0[:, :])
            b1_sb = consts.tile([H, VP], PH3_DT)
            nc.sync.dma_start(out=b1_sb, in_=beta1[:, :])
            epst_sb = consts.tile([H, 2 * B], F32)
            nc.sync.dma_start(out=epst_sb, in_=epst[:, :])
            fc1b_sb = consts.tile([H, 1], F32)
            nc.sync.dma_start(out=fc1b_sb, in_=fc1b[:, :])
            fc2w_sb = consts.tile([H, H], SN_DT)
            nc.sync.dma_start(out=fc2w_sb, in_=fc2w[:, :])
            fc2b_sb = consts.tile([H, 1], F32)
            nc.sync.dma_start(out=fc2b_sb, in_=fc2b[:, :])
            muw_sb = consts.tile([H, K], SN_DT)
            nc.sync.dma_start(out=muw_sb, in_=muw[:, :])
            sigw_sb = consts.tile([H, K], SN_DT)
            nc.sync.dma_start(out=sigw_sb, in_=sigw[:, :])
            bnmg_sb = consts.tile([H, 2], F32)
            nc.sync.dma_start(out=bnmg_sb, in_=bnmg[:, :])
            bnmb_sb = consts.tile([H, 2], F32)
            nc.sync.dma_start(out=bnmb_sb, in_=bnmb[:, :])
            bnsg_sb = consts.tile([H, 2], F32)
            nc.sync.dma_start(out=bnsg_sb, in_=bnsg[:, :])
            bnsb_sb = consts.tile([H, 2], F32)
            nc.sync.dma_start(out=bnsb_sb, in_=bnsb[:, :])
            vg_sb = consts.tile([128, NCH], F32)
            nc.sync.dma_start(out=vg_sb, in_=vg[:, :])
            vb_sb = consts.tile([128, NCH], F32)
            nc.sync.dma_start(out=vb_sb, in_=vb[:, :])
            vbb_sb = consts.tile([128, NCH], F32)
            nc.sync.dma_start(out=vbb_sb, in_=vbb[:, :])
            cnt_sb = consts.tile([128, NCH], BF16)
            nc.sync.dma_start(out=cnt_sb, in_=cnt[:, :])
            msk_sb = consts.tile([128, NCH], BF16)
            nc.sync.dma_start(out=msk_sb, in_=msk[:, :])
            epsc_h = consts.tile([H, 1], F32)
            nc.vector.memset(epsc_h, BN_EPS)
            epsc_128 = consts.tile([128, 1], F32)
            nc.vector.memset(epsc_128, BN_EPS)

            # ---- phase 1: fc1 partial over the V-slice --------------------
            sn_pool_cm = tc.tile_pool(name="sn", bufs=1)
            sn = sn_pool_cm.__enter__()
            with tc.tile_pool(name="ph1", bufs=2, space="PSUM") as ph1:
                h1a = ph1.tile([H, 512], F32, tag="h1")
                h1b = ph1.tile([H, 512], F32, tag="h1")
                n_grp = (NCH + DMA_GRP - 1) // DMA_GRP
                for d in range(n_grp):
                    j0 = d * DMA_GRP
                    nch = min(DMA_GRP, NCH - j0)
                    bt = work.tile([128, DMA_GRP * B], PH1_DT, tag="bows")
                    bt3 = bt.rearrange("p (c b) -> p c b", c=DMA_GRP)
                    nc.sync.dma_start(
                        out=bt3[:, :nch, :],
                        in_=bows_v[j0 : j0 + nch].rearrange("c p b -> p c b"),
                    )
                    for j in range(nch):
                        k = j0 + j
                        lhs = w1_sb[:, k * H : (k + 1) * H]
                        nc.tensor.matmul(
                            h1a[:, :],
                            lhs,
                            bt3[:, j, 0:512],
                            start=(k == 0),
                            stop=(k == NCH - 1),
                        )
                        nc.tensor.matmul(
                            h1b[:, :],
                            lhs,
                            bt3[:, j, 512:1024],
                            start=(k == 0),
                            stop=(k == NCH - 1),
                        )
                h1pre = sn.tile([H, B], F32)
                nc.vector.tensor_copy(h1pre[:, 0:512], h1a[:, :])
                nc.vector.tensor_copy(h1pre[:, 512:1024], h1b[:, :])
                nc.sync.dma_start(out=cc1_in[:, :], in_=h1pre)

            nc.gpsimd.collective_compute(
                "AllReduce", OP.add, replica_groups=rg,
                ins=[cc1_in[:, :]], outs=[cc1_out[:, :]],
            )

            # ---- phase 2: replicated small net ----------------------------
            har = sn.tile([H, B], F32)
            nc.sync.dma_start(out=har, in_=cc1_out[:, :])
            if TRUNC == 1:
                nc.sync.dma_start(out=loglik[:, :], in_=har[0:1, :])
                nc.compile_marker_trunc = True
            # softplus(x) = ln(1 + exp(x)); |x| < ~15 so no overflow concern.
            if TRUNC == 1:
                return _finish(nc)
            h1e = sn.tile([H, B], F32)
            nc.scalar.activation(h1e, har, AF.Exp, bias=fc1b_sb, scale=1.0)
            h1_bf = sn.tile([H, B], SN_DT)
            nc.scalar.activation(h1_bf, h1e, AF.Ln, bias=1.0, scale=1.0)

            theta_bf = consts.tile([H, 2 * B], PH3_DT)
            mun = sn.tile([H, 2 * B], F32)
            sige = sn.tile([H, 2 * B], F32)
            h2_bf = sn.tile([H, B], SN_DT)

            with tc.tile_pool(name="psn", bufs=2, space="PSUM") as psn:
                for half in range(2):
                    hps = psn.tile([H, 512], F32, tag="sn")
                    nc.tensor.matmul(
                        hps, fc2w_sb, h1_bf[:, half * 512 : (half + 1) * 512],
                        start=True, stop=True,
                    )
                    h2e = small.tile([H, 512], F32, tag="h2e", bufs=2)
                    nc.scalar.activation(h2e, hps, AF.Exp, bias=fc2b_sb, scale=1.0)
                    nc.scalar.activation(
                        h2_bf[:, half * 512 : (half + 1) * 512],
                        h2e, AF.Ln, bias=1.0, scale=1.0,
                    )

                for kh in range(2):
                    for w_sb, gg, bb_, dst, is_sig in (
                        (muw_sb, bnmg_sb, bnmb_sb, mun, False),
                        (sigw_sb, bnsg_sb, bnsb_sb, sige, True),
                    ):
                        pa = psn.tile([H, 512], F32, tag="sn")
                        pb = psn.tile([H, 512], F32, tag="sn")
                        lhs = w_sb[:, kh * H : (kh + 1) * H]
                        nc.tensor.matmul(pa, lhs, h2_bf[:, 0:512], start=True, stop=True)
                        nc.tensor.matmul(pb, lhs, h2_bf[:, 512:1024], start=True, stop=True)
                        st = small.tile([H, 2, 6], F32, tag="snst")
                        nc.vector.bn_stats(st[:, 0, :], pa)
                        nc.vector.bn_stats(st[:, 1, :], pb)
                        mv = small.tile([H, 2], F32, tag="snmv")
                        nc.vector.bn_aggr(mv, st)
                        # rstd = exp(-0.5*ln(var+eps)); keeps ACT on one table set
                        lnu = small.tile([H, 1], F32, tag="snsq")
                        nc.scalar.activation(lnu, mv[:, 1:2], AF.Ln, bias=epsc_h, scale=1.0)
                        rstd = small.tile([H, 1], F32, tag="snr")
                        nc.scalar.activation(rstd, lnu, AF.Exp, bias=0.0, scale=-0.5)
                        gr = small.tile([H, 1], F32, tag="sngr")
                        nc.vector.tensor_mul(gr, gg[:, kh : kh + 1], rstd)
                        if is_sig:
                            # sige = exp(0.5*(g*rstd*(x-m)+b)) fused on ACT
                            sc = small.tile([H, 1], F32, tag="snsc")
                            nc.vector.tensor_scalar_mul(sc, gr, 0.5)
                            bi = small.tile([H, 1], F32, tag="snbi")
                            nc.vector.tensor_mul(bi, sc, mv[:, 0:1])
                            bi2 = small.tile([H, 1], F32, tag="snbi2")
                            nc.vector.tensor_scalar_mul(bi2, bb_[:, kh : kh + 1], 0.5)
                            bi3 = small.tile([H, 1], F32, tag="snbi3")
                            nc.vector.tensor_sub(bi3, bi2, bi)
                            for half, ps in ((0, pa), (1, pb)):
                                nc.scalar.activation(
                                    dst[:, kh * B + half * 512 : kh * B + (half + 1) * 512],
                                    ps, AF.Exp, bias=bi3, scale=sc,
                                )
                        else:
                            # mun = (x - m) * (g*rstd)   (bias bnmb added later)
                            for half, ps in ((0, pa), (1, pb)):
                                nc.vector.tensor_scalar(
                                    dst[:, kh * B + half * 512 : kh * B + (half + 1) * 512],
                                    ps, mv[:, 0:1], gr, op0=OP.subtract, op1=OP.mult,
                                )

                # theta = exp(mun + bnmb + sige * epsT)
                for kh in range(2):
                    for half in range(2):
                        s = slice(kh * B + half * 512, kh * B + (half + 1) * 512)
                        tmp = small.tile([H, 512], F32, tag="snth", bufs=2)
                        nc.vector.tensor_mul(tmp, sige[:, s], epst_sb[:, s])
                        tmp2 = small.tile([H, 512], F32, tag="snth2", bufs=2)
                        nc.vector.tensor_add(tmp2, tmp, mun[:, s])
                        nc.scalar.activation(
                            theta_bf[:, s], tmp2, AF.Exp,
                            bias=bnmb_sb[:, kh : kh + 1], scale=1.0,
                        )

            sn_pool_cm.__exit__(None, None, None)

            if TRUNC == 2:
                trsb = consts.tile([1, B], F32)
                nc.vector.tensor_copy(trsb, theta_bf[0:1, 0:B])
                nc.sync.dma_start(out=loglik[:, :], in_=trsb)
                return _finish(nc)

            # ---- phase 3: beta matmul + BN + exp + reductions -------------
            with (
                tc.tile_pool(name="px", bufs=3, space="PSUM") as px,
                tc.tile_pool(name="pred", bufs=1, space="PSUM") as pred,
                tc.tile_pool(name="ph3w", bufs=1) as ph3w,
            ):
                red = pred.tile([128, 512], F32)
                PIPE = 6  # reduction matmuls trail the x matmuls by PIPE chunks
                pend = {}

                def emit_red(j):
                    ea, ya = pend.pop(j)
                    mcol = msk_sb[:, j : j + 1]
                    ccol = cnt_sb[:, j : j + 1]
                    st_ = (j == 0)
                    sp_ = (j == NCH - 1)
                    nc.tensor.matmul(red[0:1, :], mcol, ea[:, 0:512], start=st_,
                                     stop=sp_, tile_position=(0, 0))
                    nc.tensor.matmul(red[32:33, :], mcol, ea[:, 512:1024], start=st_,
                                     stop=sp_, tile_position=(0, 32))
                    nc.tensor.matmul(red[64:65, :], ccol, ya[:, 0:512], start=st_,
                                     stop=sp_, tile_position=(0, 64))
                    nc.tensor.matmul(red[96:97, :], ccol, ya[:, 512:1024], start=st_,
                                     stop=sp_, tile_position=(0, 96))

                for j in range(NCH):
                    vs = slice(j * 128, (j + 1) * 128)
                    xp = px.tile([128, 1024], F32, tag="x")
                    nc.tensor.matmul(xp[:, 0:512], b0_sb[:, vs], theta_bf[:, 0:512], start=True, stop=False)
                    nc.tensor.matmul(xp[:, 0:512], b1_sb[:, vs], theta_bf[:, B : B + 512], start=False, stop=True)
                    nc.tensor.matmul(xp[:, 512:1024], b0_sb[:, vs], theta_bf[:, 512:1024], start=True, stop=False)
                    nc.tensor.matmul(xp[:, 512:1024], b1_sb[:, vs], theta_bf[:, B + 512 : 2 * B], start=False, stop=True)

                    st = small.tile([128, 2, 6], F32, tag="xst")
                    nc.vector.bn_stats(st[:, 0, :], xp[:, 0:512])
                    nc.vector.bn_stats(st[:, 1, :], xp[:, 512:1024])
                    mv = small.tile([128, 2], F32, tag="xmv")
                    nc.vector.bn_aggr(mv, st)
                    # m2n = -mean; a = g*rsqrt(var+eps); c2 = a*m2n + bn_b
                    m2n = small.tile([128, 1], F32, tag="xm2")
                    nc.vector.tensor_scalar_mul(m2n, mv[:, 0:1], -1.0)
                    lnu = small.tile([128, 1], F32, tag="xsq")
                    nc.scalar.activation(lnu, mv[:, 1:2], AF.Ln, bias=epsc_128, scale=1.0)
                    rstd = small.tile([128, 1], F32, tag="xr")
                    nc.scalar.activation(rstd, lnu, AF.Exp, bias=0.0, scale=-0.5)
                    a = small.tile([128, 1], F32, tag="xa")
                    nc.vector.tensor_mul(a, vg_sb[:, j : j + 1], rstd)
                    c2 = small.tile([128, 1], F32, tag="xc2")
                    nc.vector.scalar_tensor_tensor(c2, a, m2n, vb_sb[:, j : j + 1],
                                                   op0=OP.mult, op1=OP.add)

                    ea = ph3w.tile([128, 1024], BF16, tag="ea", bufs=PIPE + 2)
                    nc.scalar.activation(ea, xp, AF.Exp, bias=c2, scale=a)
                    ya = ph3w.tile([128, 1024], BF16, tag="ya", bufs=PIPE + 2)
                    if YA_ON_ACT:
                        nc.scalar.activation(ya, xp, AF.Identity, bias=c2, scale=a)
                    else:
                        nc.scalar.activation(ya[:, 0:512], xp[:, 0:512], AF.Identity,
                                             bias=c2, scale=a)
                        nc.vector.tensor_scalar(ya[:, 512:1024], xp[:, 512:1024], a, c2,
                                                op0=OP.mult, op1=OP.add)
                    pend[j] = (ea, ya)
                    if j >= PIPE:
                        emit_red(j - PIPE)
                for j in range(NCH - PIPE, NCH):
                    emit_red(j)

                stg = ph3w.tile([128, 512], F32)
                nc.vector.tensor_copy(stg, red[:, :])
                if DEBUG:
                    nc.sync.dma_start(out=dbg_theta[:, :], in_=theta_bf)
                    nc.sync.dma_start(out=dbg_stg[:, :], in_=stg)
                    nc.sync.dma_start(out=dbg_h2[:, :], in_=h2_bf)
                nc.sync.dma_start(out=cc2_in[0:1, 0:512], in_=stg[0:1, :])
                nc.sync.dma_start(out=cc2_in[0:1, 512:1024], in_=stg[32:33, :])
                nc.sync.dma_start(out=cc2_in[0:1, 1024:1536], in_=stg[64:65, :])
                nc.sync.dma_start(out=cc2_in[0:1, 1536:2048], in_=stg[96:97, :])

            nc.gpsimd.collective_compute(
                "AllReduce", OP.add, replica_groups=rg,
                ins=[cc2_in[:, :]], outs=[cc2_out[:, :]],
            )

            # ---- final: loglik = S - T*ln(sumexp) -------------------------
            finp_cm = tc.tile_pool(name="finp", bufs=1)
            finp = finp_cm.__enter__()
            fin = finp.tile([1, 2 * B], F32)
            nc.sync.dma_start(out=fin, in_=cc2_out[:, :])
            if DEBUG:
                nc.sync.dma_start(out=dbg_fin[:, :], in_=fin)
            lse = finp.tile([1, B], F32)
            nc.scalar.activation(lse, fin[:, 0:B], AF.Ln, bias=0.0, scale=1.0)
            lt = finp.tile([1, B], F32)
            nc.vector.tensor_scalar_mul(lt, lse, float(T_TOK))
            res = finp.tile([1, B], F32)
            nc.vector.tensor_sub(res, fin[:, B : 2 * B], lt)
            nc.sync.dma_start(out=loglik[:, :], in_=res)
            finp_cm.__exit__(None, None, None)

    return _finish(nc)


def _finish(nc):
    nc.compile()
    return nc


_NC_CACHE = None


def _get_nc():
    global _NC_CACHE
    if _NC_CACHE is None:
        _NC_CACHE = _build_nc()
    return _NC_CACHE


def _np_dt(mdt):
    import concourse.mybir as _mb
    return ml_dtypes.bfloat16 if mdt == _mb.dt.bfloat16 else np.float32


def _prep_core_inputs(c, bows, eps, fc1_w, fc1_b, fc2_w, fc2_b, mu_w,
                      bn_mu_g, bn_mu_b, sig_w, bn_sig_g, bn_sig_b,
                      beta_w, beta_b, bn_beta_g, bn_beta_b, counts):
    sl = slice(c * VS, (c + 1) * VS)

    def padv(x, fill=0.0):
        out = np.full(VP, fill, np.float32)
        out[:VS] = x[sl]
        return out

    bowsT = np.zeros((VP, B), np.float32)
    bowsT[:VS] = bows[:, sl].T
    w1 = np.zeros((VP, H), np.float32)
    w1[:VS] = fc1_w[sl]

    vcol = lambda x: np.ascontiguousarray(x.reshape(NCH, 128).T)

    return {
        "bowsT": bowsT.astype(_np_dt(PH1_DT)),
        "w1": np.ascontiguousarray(
            w1.reshape(NCH, 128, H).transpose(1, 0, 2).reshape(128, NCH * H)
        ).astype(_np_dt(PH1_DT)),
        "beta0": np.ascontiguousarray(
            np.pad(beta_w[0:H, sl], ((0, 0), (0, VP - VS)))
        ).astype(_np_dt(PH3_DT)),
        "beta1": np.ascontiguousarray(
            np.pad(beta_w[H:K, sl], ((0, 0), (0, VP - VS)))
        ).astype(_np_dt(PH3_DT)),
        "epst": np.ascontiguousarray(
            eps.T.reshape(2, H, B).transpose(1, 0, 2).reshape(H, 2 * B)
        ).astype(np.float32),
        "fc1b": fc1_b.reshape(H, 1).astype(np.float32),
        "fc2w": fc2_w.astype(_np_dt(SN_DT)),
        "fc2b": fc2_b.reshape(H, 1).astype(np.float32),
        "muw": np.ascontiguousarray(mu_w.reshape(H, K)).astype(_np_dt(SN_DT)),
        "sigw": np.ascontiguousarray(sig_w.reshape(H, K)).astype(_np_dt(SN_DT)),
        "bnmg": np.ascontiguousarray(bn_mu_g.reshape(2, H).T).astype(np.float32),
        "bnmb": np.ascontiguousarray(bn_mu_b.reshape(2, H).T).astype(np.float32),
        "bnsg": np.ascontiguousarray(bn_sig_g.reshape(2, H).T).astype(np.float32),
        "bnsb": np.ascontiguousarray(bn_sig_b.reshape(2, H).T).astype(np.float32),
        "vg": vcol(padv(bn_beta_g, 1.0)),
        "vb": vcol(padv(bn_beta_b)),
        "vbb": vcol(padv(beta_b)),
        "cnt": vcol(padv(counts)).astype(ml_dtypes.bfloat16),
        "msk": vcol(np.pad(np.ones(VS, np.float32), (0, VP - VS))).astype(ml_dtypes.bfloat16),
    }


def kernel(bows, eps, ne_tokens, fc1_w, fc1_b, fc2_w, fc2_b,
           mu_w, mu_b, bn_mu_g, bn_mu_b, sig_w, sig_b, bn_sig_g, bn_sig_b,
           beta_w, beta_b, bn_beta_g, bn_beta_b):
    bows = np.asarray(bows, np.float32)
    eps = np.asarray(eps, np.float32)
    counts = np.bincount(np.asarray(ne_tokens), minlength=V).astype(np.float32)

    # mu_b / sig_b cancel inside BatchNorm (shift-invariant); not shipped.
    args = (bows, eps, np.asarray(fc1_w, np.float32), np.asarray(fc1_b, np.float32),
            np.asarray(fc2_w, np.float32), np.asarray(fc2_b, np.float32),
            np.asarray(mu_w, np.float32), np.asarray(bn_mu_g, np.float32),
            np.asarray(bn_mu_b, np.float32), np.asarray(sig_w, np.float32),
            np.asarray(bn_sig_g, np.float32), np.asarray(bn_sig_b, np.float32),
            np.asarray(beta_w, np.float32), np.asarray(beta_b, np.float32),
            np.asarray(bn_beta_g, np.float32), np.asarray(bn_beta_b, np.float32),
            counts)

    in_maps = [_prep_core_inputs(c, *args) for c in range(N_CORES)]
    global _last_in_maps
    _last_in_maps = in_maps
    nc = _get_nc()
    res = run_bass_kernel_spmd(nc, in_maps, list(range(N_CORES)))
    return np.asarray(res.results[0]["loglik"]).reshape(B).astype(np.float32)


_last_in_maps = None


# revision 34
# speedup vs baseline: 1.1236x; 1.0616x over previous
"""NeLDA loglik kernel for 8 Trainium2 NeuronCores (Bass/Tile).

Strategy: vocab-parallel over V=50000 (6250 cols/core, padded to 6272).
  phase 1: per-core partial fc1 (contraction over its V-slice), AllReduce
           the [H=100, B=1024] pre-activation.
  phase 2: replicated small net (fc2, mu/sig BatchNorm, theta) in
           transposed [feature, batch] layout.
  phase 3: per-core logits slice x = theta @ beta_w[:, slice] (in PSUM,
           [v, b] layout), BatchNorm stats per v-row via bn_stats,
           exp fused with BN via ScalarE per-partition scale/bias,
           partition-dir reductions (sum_v exp, sum_v cnt*y) via
           ones/counts matmuls packed on PE column strips.
  final:   AllReduce [sumexp | S] (8KB), loglik = S - T*ln(sumexp).

The token gather sum_t log_beta[:, tok_t] is rewritten as a count-weighted
sum over the vocab: S[b] = sum_v cnt_v * y[v, b], cnt = histogram(tokens).
"""

import ml_dtypes
import numpy as np

import concourse.bass as bass
import concourse.mybir as mybir
import concourse.tile as tile
from concourse import bacc
from concourse.bass_utils import run_bass_kernel_spmd

F32 = mybir.dt.float32
F32R = mybir.dt.float32r
BF16 = mybir.dt.bfloat16
AF = mybir.ActivationFunctionType
OP = mybir.AluOpType

N_CORES = 8
B = 1024
V = 50000
H = 100
K = 200
T_TOK = 32768
BN_EPS = 1e-5

DEBUG = False
YA_SPLIT = 1024  # cols of the y-pass done on ScalarE; rest on VectorE
TRUNC = 0  # 1: stop after AR1, 2: stop after small net
# f32r (TF32-like, full-rate) for the big beta matmul; exact f32 for fc1 and
# the small net (fc1 in f32r triggers a hardware fault — see notes).
PH1_DT = F32
PH3_DT = F32R
SN_DT = F32

VS = V // N_CORES          # 6250 per-core vocab slice
NCH = (VS + 127) // 128    # 49 chunks of 128 v-rows
VP = NCH * 128             # 6272 padded
DMA_GRP = 2                # bows chunks per DMA (1 MiB each)


def _build_nc():
    nc = bacc.Bacc("TRN2", target_bir_lowering=False, num_devices=N_CORES)

    bowsh = nc.dram_tensor("bowsh", [VP, B], BF16, kind="ExternalInput")
    bowsl = nc.dram_tensor("bowsl", [VP, B], BF16, kind="ExternalInput")
    w1h = nc.dram_tensor("w1h", [128, NCH * H], BF16, kind="ExternalInput")
    w1l = nc.dram_tensor("w1l", [128, NCH * H], BF16, kind="ExternalInput")
    beta0 = nc.dram_tensor("beta0", [H, VP], PH3_DT, kind="ExternalInput")
    beta1 = nc.dram_tensor("beta1", [H, VP], PH3_DT, kind="ExternalInput")
    epst = nc.dram_tensor("epst", [H, 2 * B], F32, kind="ExternalInput")
    fc1b = nc.dram_tensor("fc1b", [H, 1], F32, kind="ExternalInput")
    fc2w = nc.dram_tensor("fc2w", [H, H], SN_DT, kind="ExternalInput")
    fc2b = nc.dram_tensor("fc2b", [H, 1], F32, kind="ExternalInput")
    muw = nc.dram_tensor("muw", [H, K], SN_DT, kind="ExternalInput")
    sigw = nc.dram_tensor("sigw", [H, K], SN_DT, kind="ExternalInput")
    bnmg = nc.dram_tensor("bnmg", [H, 2], F32, kind="ExternalInput")
    bnmb = nc.dram_tensor("bnmb", [H, 2], F32, kind="ExternalInput")
    bnsg = nc.dram_tensor("bnsg", [H, 2], F32, kind="ExternalInput")
    bnsb = nc.dram_tensor("bnsb", [H, 2], F32, kind="ExternalInput")
    vg = nc.dram_tensor("vg", [128, NCH], F32, kind="ExternalInput")
    vb = nc.dram_tensor("vb", [128, NCH], F32, kind="ExternalInput")
    vbb = nc.dram_tensor("vbb", [128, NCH], F32, kind="ExternalInput")
    cnt = nc.dram_tensor("cnt", [128, NCH], BF16, kind="ExternalInput")
    msk = nc.dram_tensor("msk", [128, NCH], BF16, kind="ExternalInput")

    loglik = nc.dram_tensor("loglik", [1, B], F32, kind="ExternalOutput")
    if DEBUG:
        dbg_theta = nc.dram_tensor("dbg_theta", [H, 2 * B], F32, kind="ExternalOutput")
        dbg_stg = nc.dram_tensor("dbg_stg", [128, 512], F32, kind="ExternalOutput")
        dbg_fin = nc.dram_tensor("dbg_fin", [1, 2 * B], F32, kind="ExternalOutput")
        dbg_h2 = nc.dram_tensor("dbg_h2", [H, B], F32, kind="ExternalOutput")

    cc1_in = nc.dram_tensor("cc1_in", [H, B], F32)
    cc1_out = nc.dram_tensor("cc1_out", [H, B], F32, addr_space="Shared")
    cc2_in = nc.dram_tensor("cc2_in", [1, 2 * B], F32)
    cc2_out = nc.dram_tensor("cc2_out", [1, 2 * B], F32, addr_space="Shared")

    rg = [list(range(N_CORES))]

    bowsh_v = bowsh.rearrange("(c p) b -> c p b", p=128)  # [NCH, 128, B]
    bowsl_v = bowsl.rearrange("(c p) b -> c p b", p=128)

    with tile.TileContext(nc) as tc:
        with (
            tc.tile_pool(name="consts", bufs=1) as consts,
            tc.tile_pool(name="work", bufs=3) as work,
            tc.tile_pool(name="small", bufs=4) as small,
        ):
            # ---- resident loads -------------------------------------------
            w1h_sb = consts.tile([128, NCH * H], BF16)
            nc.sync.dma_start(out=w1h_sb, in_=w1h[:, :])
            w1l_sb = consts.tile([128, NCH * H], BF16)
            nc.sync.dma_start(out=w1l_sb, in_=w1l[:, :])
            b0_sb = consts.tile([H, VP], PH3_DT)
            nc.sync.dma_start(out=b0_sb, in_=beta0[:, :])
            b1_sb = consts.tile([H, VP], PH3_DT)
            nc.sync.dma_start(out=b1_sb, in_=beta1[:, :])
            epst_sb = consts.tile([H, 2 * B], F32)
            nc.sync.dma_start(out=epst_sb, in_=epst[:, :])
            fc1b_sb = consts.tile([H, 1], F32)
            nc.sync.dma_start(out=fc1b_sb, in_=fc1b[:, :])
            fc2w_sb = consts.tile([H, H], SN_DT)
            nc.sync.dma_start(out=fc2w_sb, in_=fc2w[:, :])
            fc2b_sb = consts.tile([H, 1], F32)
            nc.sync.dma_start(out=fc2b_sb, in_=fc2b[:, :])
            muw_sb = consts.tile([H, K], SN_DT)
            nc.sync.dma_start(out=muw_sb, in_=muw[:, :])
            sigw_sb = consts.tile([H, K], SN_DT)
            nc.sync.dma_start(out=sigw_sb, in_=sigw[:, :])
            bnmg_sb = consts.tile([H, 2], F32)
            nc.sync.dma_start(out=bnmg_sb, in_=bnmg[:, :])
            bnmb_sb = consts.tile([H, 2], F32)
            nc.sync.dma_start(out=bnmb_sb, in_=bnmb[:, :])
            bnsg_sb = consts.tile([H, 2], F32)
            nc.sync.dma_start(out=bnsg_sb, in_=bnsg[:, :])
            bnsb_sb = consts.tile([H, 2], F32)
            nc.sync.dma_start(out=bnsb_sb, in_=bnsb[:, :])
            vg_sb = consts.tile([128, NCH], F32)
            nc.sync.dma_start(out=vg_sb, in_=vg[:, :])
            vb_sb = consts.tile([128, NCH], F32)
            nc.sync.dma_start(out=vb_sb, in_=vb[:, :])
            vbb_sb = consts.tile([128, NCH], F32)
            nc.sync.dma_start(out=vbb_sb, in_=vbb[:, :])
            cnt_sb = consts.tile([128, NCH], BF16)
            nc.sync.dma_start(out=cnt_sb, in_=cnt[:, :])
            msk_sb = consts.tile([128, NCH], BF16)
            nc.sync.dma_start(out=msk_sb, in_=msk[:, :])
            epsc_h = consts.tile([H, 1], F32)
            nc.vector.memset(epsc_h, BN_EPS)
            epsc_128 = consts.tile([128, 1], F32)
            nc.vector.memset(epsc_128, BN_EPS)

            # ---- phase 1: fc1 partial over the V-slice --------------------
            sn_pool_cm = tc.tile_pool(name="sn", bufs=1)
            sn = sn_pool_cm.__enter__()
            with tc.tile_pool(name="ph1", bufs=2, space="PSUM") as ph1:
                h1a = ph1.tile([H, 512], F32, tag="h1")
                h1b = ph1.tile([H, 512], F32, tag="h1")
                n_grp = (NCH + DMA_GRP - 1) // DMA_GRP
                for d in range(n_grp):
                    j0 = d * DMA_GRP
                    nch = min(DMA_GRP, NCH - j0)
                    bth = work.tile([128, DMA_GRP * B], BF16, tag="bowsh")
                    bth3 = bth.rearrange("p (c b) -> p c b", c=DMA_GRP)
                    nc.sync.dma_start(
                        out=bth3[:, :nch, :],
                        in_=bowsh_v[j0 : j0 + nch].rearrange("c p b -> p c b"),
                    )
                    btl = work.tile([128, DMA_GRP * B], BF16, tag="bowsl")
                    btl3 = btl.rearrange("p (c b) -> p c b", c=DMA_GRP)
                    nc.sync.dma_start(
                        out=btl3[:, :nch, :],
                        in_=bowsl_v[j0 : j0 + nch].rearrange("c p b -> p c b"),
                    )
                    for j in range(nch):
                        k = j0 + j
                        lh = w1h_sb[:, k * H : (k + 1) * H]
                        ll = w1l_sb[:, k * H : (k + 1) * H]
                        st_ = (k == 0)
                        sp_ = (k == NCH - 1)
                        for half, sl in ((h1a, slice(0, 512)), (h1b, slice(512, 1024))):
                            nc.tensor.matmul(half[:, :], lh, bth3[:, j, sl],
                                             start=st_, stop=False)
                            nc.tensor.matmul(half[:, :], ll, bth3[:, j, sl],
                                             start=False, stop=False)
                            nc.tensor.matmul(half[:, :], lh, btl3[:, j, sl],
                                             start=False, stop=sp_)
                h1pre = sn.tile([H, B], F32)
                nc.vector.tensor_copy(h1pre[:, 0:512], h1a[:, :])
                nc.vector.tensor_copy(h1pre[:, 512:1024], h1b[:, :])
                nc.sync.dma_start(out=cc1_in[:, :], in_=h1pre)

            nc.gpsimd.collective_compute(
                "AllReduce", OP.add, replica_groups=rg,
                ins=[cc1_in[:, :]], outs=[cc1_out[:, :]],
            )

            # ---- phase 2: replicated small net ----------------------------
            har = sn.tile([H, B], F32)
            nc.sync.dma_start(out=har, in_=cc1_out[:, :])
            if TRUNC == 1:
                nc.sync.dma_start(out=loglik[:, :], in_=har[0:1, :])
                nc.compile_marker_trunc = True
            # softplus(x) = ln(1 + exp(x)); |x| < ~15 so no overflow concern.
            if TRUNC == 1:
                return _finish(nc)
            h1e = sn.tile([H, B], F32)
            nc.scalar.activation(h1e, har, AF.Exp, bias=fc1b_sb, scale=1.0)
            h1_bf = sn.tile([H, B], SN_DT)
            nc.scalar.activation(h1_bf, h1e, AF.Ln, bias=1.0, scale=1.0)

            theta_bf = consts.tile([H, 2 * B], PH3_DT)
            mun = sn.tile([H, 2 * B], F32)
            sige = sn.tile([H, 2 * B], F32)
            h2_bf = sn.tile([H, B], SN_DT)

            with tc.tile_pool(name="psn", bufs=2, space="PSUM") as psn:
                for half in range(2):
                    hps = psn.tile([H, 512], F32, tag="sn")
                    nc.tensor.matmul(
                        hps, fc2w_sb, h1_bf[:, half * 512 : (half + 1) * 512],
                        start=True, stop=True,
                    )
                    h2e = small.tile([H, 512], F32, tag="h2e", bufs=2)
                    nc.scalar.activation(h2e, hps, AF.Exp, bias=fc2b_sb, scale=1.0)
                    nc.scalar.activation(
                        h2_bf[:, half * 512 : (half + 1) * 512],
                        h2e, AF.Ln, bias=1.0, scale=1.0,
                    )

                for kh in range(2):
                    for w_sb, gg, bb_, dst, is_sig in (
                        (muw_sb, bnmg_sb, bnmb_sb, mun, False),
                        (sigw_sb, bnsg_sb, bnsb_sb, sige, True),
                    ):
                        pa = psn.tile([H, 512], F32, tag="sn")
                        pb = psn.tile([H, 512], F32, tag="sn")
                        lhs = w_sb[:, kh * H : (kh + 1) * H]
                        nc.tensor.matmul(pa, lhs, h2_bf[:, 0:512], start=True, stop=True)
                        nc.tensor.matmul(pb, lhs, h2_bf[:, 512:1024], start=True, stop=True)
                        st = small.tile([H, 2, 6], F32, tag="snst")
                        nc.vector.bn_stats(st[:, 0, :], pa)
                        nc.vector.bn_stats(st[:, 1, :], pb)
                        mv = small.tile([H, 2], F32, tag="snmv")
                        nc.vector.bn_aggr(mv, st)
                        # rstd = exp(-0.5*ln(var+eps)); keeps ACT on one table set
                        lnu = small.tile([H, 1], F32, tag="snsq")
                        nc.scalar.activation(lnu, mv[:, 1:2], AF.Ln, bias=epsc_h, scale=1.0)
                        rstd = small.tile([H, 1], F32, tag="snr")
                        nc.scalar.activation(rstd, lnu, AF.Exp, bias=0.0, scale=-0.5)
                        gr = small.tile([H, 1], F32, tag="sngr")
                        nc.vector.tensor_mul(gr, gg[:, kh : kh + 1], rstd)
                        if is_sig:
                            # sige = exp(0.5*(g*rstd*(x-m)+b)) fused on ACT
                            sc = small.tile([H, 1], F32, tag="snsc")
                            nc.vector.tensor_scalar_mul(sc, gr, 0.5)
                            bi = small.tile([H, 1], F32, tag="snbi")
                            nc.vector.tensor_mul(bi, sc, mv[:, 0:1])
                            bi2 = small.tile([H, 1], F32, tag="snbi2")
                            nc.vector.tensor_scalar_mul(bi2, bb_[:, kh : kh + 1], 0.5)
                            bi3 = small.tile([H, 1], F32, tag="snbi3")
                            nc.vector.tensor_sub(bi3, bi2, bi)
                            for half, ps in ((0, pa), (1, pb)):
                                nc.scalar.activation(
                                    dst[:, kh * B + half * 512 : kh * B + (half + 1) * 512],
                                    ps, AF.Exp, bias=bi3, scale=sc,
                                )
                        else:
                            # mun = (x - m) * (g*rstd)   (bias bnmb added later)
                            for half, ps in ((0, pa), (1, pb)):
                                nc.vector.tensor_scalar(
                                    dst[:, kh * B + half * 512 : kh * B + (half + 1) * 512],
                                    ps, mv[:, 0:1], gr, op0=OP.subtract, op1=OP.mult,
                                )

                # theta = exp(mun + bnmb + sige * epsT)
                for kh in range(2):
                    for half in range(2):
                        s = slice(kh * B + half * 512, kh * B + (half + 1) * 512)
                        tmp = small.tile([H, 512], F32, tag="snth", bufs=2)
                        nc.vector.tensor_mul(tmp, sige[:, s], epst_sb[:, s])
                        tmp2 = small.tile([H, 512], F32, tag="snth2", bufs=2)
                        nc.vector.tensor_add(tmp2, tmp, mun[:, s])
                        nc.scalar.activation(
                            theta_bf[:, s], tmp2, AF.Exp,
                            bias=bnmb_sb[:, kh : kh + 1], scale=1.0,
                        )

            sn_pool_cm.__exit__(None, None, None)

            if TRUNC == 2:
                trsb = consts.tile([1, B], F32)
                nc.vector.tensor_copy(trsb, theta_bf[0:1, 0:B])
                nc.sync.dma_start(out=loglik[:, :], in_=trsb)
                return _finish(nc)

            # ---- phase 3: beta matmul + BN + exp + reductions -------------
            with (
                tc.tile_pool(name="px", bufs=3, space="PSUM") as px,
                tc.tile_pool(name="pred", bufs=1, space="PSUM") as pred,
                tc.tile_pool(name="ph3w", bufs=1) as ph3w,
            ):
                red = pred.tile([128, 512], F32)
                PIPE = 6  # reduction matmuls trail the x matmuls by PIPE chunks
                pend = {}

                def emit_red(j):
                    ea, ya = pend.pop(j)
                    mcol = msk_sb[:, j : j + 1]
                    ccol = cnt_sb[:, j : j + 1]
                    st_ = (j == 0)
                    sp_ = (j == NCH - 1)
                    nc.tensor.matmul(red[0:1, :], mcol, ea[:, 0:512], start=st_,
                                     stop=sp_, tile_position=(0, 0))
                    nc.tensor.matmul(red[32:33, :], mcol, ea[:, 512:1024], start=st_,
                                     stop=sp_, tile_position=(0, 32))
                    nc.tensor.matmul(red[64:65, :], ccol, ya[:, 0:512], start=st_,
                                     stop=sp_, tile_position=(0, 64))
                    nc.tensor.matmul(red[96:97, :], ccol, ya[:, 512:1024], start=st_,
                                     stop=sp_, tile_position=(0, 96))

                for j in range(NCH):
                    vs = slice(j * 128, (j + 1) * 128)
                    xp = px.tile([128, 1024], F32, tag="x")
                    nc.tensor.matmul(xp[:, 0:512], b0_sb[:, vs], theta_bf[:, 0:512], start=True, stop=False)
                    nc.tensor.matmul(xp[:, 0:512], b1_sb[:, vs], theta_bf[:, B : B + 512], start=False, stop=True)
                    nc.tensor.matmul(xp[:, 512:1024], b0_sb[:, vs], theta_bf[:, 512:1024], start=True, stop=False)
                    nc.tensor.matmul(xp[:, 512:1024], b1_sb[:, vs], theta_bf[:, B + 512 : 2 * B], start=False, stop=True)

                    st = small.tile([128, 2, 6], F32, tag="xst")
                    nc.vector.bn_stats(st[:, 0, :], xp[:, 0:512])
                    nc.vector.bn_stats(st[:, 1, :], xp[:, 512:1024])
                    mv = small.tile([128, 2], F32, tag="xmv")
                    nc.vector.bn_aggr(mv, st)
                    # m2n = -mean; a = g*rsqrt(var+eps); c2 = a*m2n + bn_b
                    m2n = small.tile([128, 1], F32, tag="xm2")
                    nc.vector.tensor_scalar_mul(m2n, mv[:, 0:1], -1.0)
                    lnu = small.tile([128, 1], F32, tag="xsq")
                    nc.scalar.activation(lnu, mv[:, 1:2], AF.Ln, bias=epsc_128, scale=1.0)
                    rstd = small.tile([128, 1], F32, tag="xr")
                    nc.scalar.activation(rstd, lnu, AF.Exp, bias=0.0, scale=-0.5)
                    a = small.tile([128, 1], F32, tag="xa")
                    nc.vector.tensor_mul(a, vg_sb[:, j : j + 1], rstd)
                    c2 = small.tile([128, 1], F32, tag="xc2")
                    nc.vector.scalar_tensor_tensor(c2, a, m2n, vb_sb[:, j : j + 1],
                                                   op0=OP.mult, op1=OP.add)

                    ea = ph3w.tile([128, 1024], BF16, tag="ea", bufs=PIPE + 2)
                    nc.scalar.activation(ea, xp, AF.Exp, bias=c2, scale=a)
                    ya = ph3w.tile([128, 1024], BF16, tag="ya", bufs=PIPE + 2)
                    if YA_SPLIT >= 1024:
                        nc.scalar.activation(ya, xp, AF.Identity, bias=c2, scale=a)
                    else:
                        nc.scalar.activation(ya[:, 0:YA_SPLIT], xp[:, 0:YA_SPLIT],
                                             AF.Identity, bias=c2, scale=a)
                        nc.vector.tensor_scalar(ya[:, YA_SPLIT:1024], xp[:, YA_SPLIT:1024],
                                                a, c2, op0=OP.mult, op1=OP.add)
                    pend[j] = (ea, ya)
                    if j >= PIPE:
                        emit_red(j - PIPE)
                for j in range(NCH - PIPE, NCH):
                    emit_red(j)

                stg = ph3w.tile([128, 512], F32)
                nc.vector.tensor_copy(stg, red[:, :])
                if DEBUG:
                    nc.sync.dma_start(out=dbg_theta[:, :], in_=theta_bf)
                    nc.sync.dma_start(out=dbg_stg[:, :], in_=stg)
                    nc.sync.dma_start(out=dbg_h2[:, :], in_=h2_bf)
                nc.sync.dma_start(out=cc2_in[0:1, 0:512], in_=stg[0:1, :])
                nc.sync.dma_start(out=cc2_in[0:1, 512:1024], in_=stg[32:33, :])
                nc.sync.dma_start(out=cc2_in[0:1, 1024:1536], in_=stg[64:65, :])
                nc.sync.dma_start(out=cc2_in[0:1, 1536:2048], in_=stg[96:97, :])

            nc.gpsimd.collective_compute(
                "AllReduce", OP.add, replica_groups=rg,
                ins=[cc2_in[:, :]], outs=[cc2_out[:, :]],
            )

            # ---- final: loglik = S - T*ln(sumexp) -------------------------
            finp_cm = tc.tile_pool(name="finp", bufs=1)
            finp = finp_cm.__enter__()
            fin = finp.tile([1, 2 * B], F32)
            nc.sync.dma_start(out=fin, in_=cc2_out[:, :])
            if DEBUG:
                nc.sync.dma_start(out=dbg_fin[:, :], in_=fin)
            lse = finp.tile([1, B], F32)
            nc.scalar.activation(lse, fin[:, 0:B], AF.Ln, bias=0.0, scale=1.0)
            lt = finp.tile([1, B], F32)
            nc.vector.tensor_scalar_mul(lt, lse, float(T_TOK))
            res = finp.tile([1, B], F32)
            nc.vector.tensor_sub(res, fin[:, B : 2 * B], lt)
            nc.sync.dma_start(out=loglik[:, :], in_=res)
            finp_cm.__exit__(None, None, None)

    return _finish(nc)


def _finish(nc):
    nc.compile()
    return nc


_NC_CACHE = None


def _get_nc():
    global _NC_CACHE
    if _NC_CACHE is None:
        _NC_CACHE = _build_nc()
    return _NC_CACHE


def _np_dt(mdt):
    import concourse.mybir as _mb
    return ml_dtypes.bfloat16 if mdt == _mb.dt.bfloat16 else np.float32


def _prep_core_inputs(c, bows, eps, fc1_w, fc1_b, fc2_w, fc2_b, mu_w,
                      bn_mu_g, bn_mu_b, sig_w, bn_sig_g, bn_sig_b,
                      beta_w, beta_b, bn_beta_g, bn_beta_b, counts):
    sl = slice(c * VS, (c + 1) * VS)

    def padv(x, fill=0.0):
        out = np.full(VP, fill, np.float32)
        out[:VS] = x[sl]
        return out

    bowsT = np.zeros((VP, B), np.float32)
    bowsT[:VS] = bows[:, sl].T
    bh = bowsT.astype(ml_dtypes.bfloat16)
    bl = (bowsT - bh.astype(np.float32)).astype(ml_dtypes.bfloat16)
    w1 = np.zeros((VP, H), np.float32)
    w1[:VS] = fc1_w[sl]
    w1h_ = w1.astype(ml_dtypes.bfloat16)
    w1l_ = (w1 - w1h_.astype(np.float32)).astype(ml_dtypes.bfloat16)

    def w1pack(x):
        return np.ascontiguousarray(
            x.reshape(NCH, 128, H).transpose(1, 0, 2).reshape(128, NCH * H))

    vcol = lambda x: np.ascontiguousarray(x.reshape(NCH, 128).T)

    return {
        "bowsh": bh,
        "bowsl": bl,
        "w1h": w1pack(w1h_),
        "w1l": w1pack(w1l_),
        "beta0": np.ascontiguousarray(
            np.pad(beta_w[0:H, sl], ((0, 0), (0, VP - VS)))
        ).astype(_np_dt(PH3_DT)),
        "beta1": np.ascontiguousarray(
            np.pad(beta_w[H:K, sl], ((0, 0), (0, VP - VS)))
        ).astype(_np_dt(PH3_DT)),
        "epst": np.ascontiguousarray(
            eps.T.reshape(2, H, B).transpose(1, 0, 2).reshape(H, 2 * B)
        ).astype(np.float32),
        "fc1b": fc1_b.reshape(H, 1).astype(np.float32),
        "fc2w": fc2_w.astype(_np_dt(SN_DT)),
        "fc2b": fc2_b.reshape(H, 1).astype(np.float32),
        "muw": np.ascontiguousarray(mu_w.reshape(H, K)).astype(_np_dt(SN_DT)),
        "sigw": np.ascontiguousarray(sig_w.reshape(H, K)).astype(_np_dt(SN_DT)),
        "bnmg": np.ascontiguousarray(bn_mu_g.reshape(2, H).T).astype(np.float32),
        "bnmb": np.ascontiguousarray(bn_mu_b.reshape(2, H).T).astype(np.float32),
        "bnsg": np.ascontiguousarray(bn_sig_g.reshape(2, H).T).astype(np.float32),
        "bnsb": np.ascontiguousarray(bn_sig_b.reshape(2, H).T).astype(np.float32),
        "vg": vcol(padv(bn_beta_g, 1.0)),
        "vb": vcol(padv(bn_beta_b)),
        "vbb": vcol(padv(beta_b)),
        "cnt": vcol(padv(counts)).astype(ml_dtypes.bfloat16),
        "msk": vcol(np.pad(np.ones(VS, np.float32), (0, VP - VS))).astype(ml_dtypes.bfloat16),
    }


def kernel(bows, eps, ne_tokens, fc1_w, fc1_b, fc2_w, fc2_b,
           mu_w, mu_b, bn_mu_g, bn_mu_b, sig_w, sig_b, bn_sig_g, bn_sig_b,
           beta_w, beta_b, bn_beta_g, bn_beta_b):
    bows = np.asarray(bows, np.float32)
    eps = np.asarray(eps, np.float32)
    counts = np.bincount(np.asarray(ne_tokens), minlength=V).astype(np.float32)

    # mu_b / sig_b cancel inside BatchNorm (shift-invariant); not shipped.
    args = (bows, eps, np.asarray(fc1_w, np.float32), np.asarray(fc1_b, np.float32),
            np.asarray(fc2_w, np.float32), np.asarray(fc2_b, np.float32),
            np.asarray(mu_w, np.float32), np.asarray(bn_mu_g, np.float32),
            np.asarray(bn_mu_b, np.float32), np.asarray(sig_w, np.float32),
            np.asarray(bn_sig_g, np.float32), np.asarray(bn_sig_b, np.float32),
            np.asarray(beta_w, np.float32), np.asarray(beta_b, np.float32),
            np.asarray(bn_beta_g, np.float32), np.asarray(bn_beta_b, np.float32),
            counts)

    in_maps = [_prep_core_inputs(c, *args) for c in range(N_CORES)]
    global _last_in_maps
    _last_in_maps = in_maps
    nc = _get_nc()
    res = run_bass_kernel_spmd(nc, in_maps, list(range(N_CORES)))
    return np.asarray(res.results[0]["loglik"]).reshape(B).astype(np.float32)


_last_in_maps = None
